# revision 13
# baseline (speedup 1.0000x reference)
"""Multi-head attention (QKV proj + RoPE + softmax attention + output proj)
for Trainium2, tensor-parallel over heads across 8 NeuronCores.

Shapes (hardcoded): hidden_states [2, 2048, 2048], 16 heads x 128 head_dim.
Each core computes 2 heads end-to-end:
  q/k/v column-sharded projections -> RoPE -> scores^T -> exp (no max-sub,
  scores are ~+-7) -> denominator via running DVE adds + gpsimd partition
  all-reduce -> out^T = v^T @ exp^T -> normalize -> row-sharded O-projection
  partial, DMA'd straight from PSUM. Host sums the 8 partial outputs.

Schedule: phase A (projections+RoPE) for both batches, then per (b, qb)
"groups": the two B-units (h=0,1) are interleaved kt-step-wise with the
C-group (O-projection) lagging two groups behind, keeping PE saturated while
ACT runs exp and DVE runs the softmax-denominator adds.

Device layouts:
  - X^T [2048 hidden, 4096 tokens] streamed as fp32r (full-rate PE).
  - q^T/k^T kept [128 d, tokens] per head (contraction on partitions).
  - v kept token-major [tokens, 256] (keys on partitions for out^T matmul).
  - RoPE via sign-folded permutation matmul: tmp = S @ q, then
    q_rot = q*cos + tmp*sin elementwise on DVE.
  - expT ring of 4 kt-slots [128, 4, 512] bf16 (consumers lag <= 1 kt).
"""

import math

import numpy as np

HIDDEN = 2048
NH = 16
HD = 128
B = 2
S = 2048
T = B * S
NCORES = 8
HPC = NH // NCORES  # heads per core
CW = HPC * HD  # per-core projection width (256)
BASE = 10000.0
TB = 256  # phase-A token block
QB = 512  # phase-B query block
NKT = S // 128  # key tiles per batch (16)
NCH = HIDDEN // 128  # contraction chunks (16)
NQB = S // QB  # query blocks per batch (4)
NTT = QB // 128  # token tiles per query block (4)
NNB = HIDDEN // QB  # output column blocks (4)

_CACHE = {}


def _kernel_body(tc, aps, repeat=1, phases="ABC"):
    import concourse.bass as bass  # noqa: F401
    import concourse.bass_isa as bass_isa
    from concourse import mybir

    nc = tc.nc
    f32 = mybir.dt.float32
    f32r = mybir.dt.float32r
    bf16 = mybir.dt.bfloat16
    Act = mybir.ActivationFunctionType

    xt_r = aps["xt"].rearrange("(c p) t -> p c t", p=128)
    wq_r = aps["wq"].rearrange("(c p) m -> p c m", p=128)
    wk_r = aps["wk"].rearrange("(c p) m -> p c m", p=128)
    wv_r = aps["wv"].rearrange("(c p) m -> p c m", p=128)
    wo_r = aps["wo"].rearrange("(h p) n -> p h n", p=128)
    out_ap = aps["out"]

    qscale = 1.0 / math.sqrt(HD)

    with (
        tc.tile_pool(name="consts", bufs=1) as consts,
        tc.tile_pool(name="big", bufs=2) as big,
        tc.tile_pool(name="xt", bufs=2) as xtp,
        tc.tile_pool(name="rope", bufs=6) as rope,
        tc.tile_pool(name="expp", bufs=2) as expp,
        tc.tile_pool(name="small", bufs=2) as small,
        tc.tile_pool(name="stage", bufs=2) as stagep,
        tc.tile_pool(name="ps", bufs=2, space="PSUM") as psp,
        tc.tile_pool(name="pso", bufs=2, space="PSUM") as psob,
        tc.tile_pool(name="psn", bufs=2, space="PSUM") as psnp,
    ):
        # ---- constants ----
        wq_sb = consts.tile([128, NCH, CW], f32r, tag="wq")
        wk_sb = consts.tile([128, NCH, CW], f32r, tag="wk")
        wv_sb = consts.tile([128, NCH, CW], f32r, tag="wv")
        wo_sb = consts.tile([128, HPC, HIDDEN], bf16, tag="wo")
        cos_sb = consts.tile([128, S], bf16, tag="cos")
        sin_sb = consts.tile([128, S], bf16, tag="sin")
        st_sb = consts.tile([128, 128], bf16, tag="st")
        ones_sb = consts.tile([128, 128], bf16, tag="ones")
        bqk_sb = consts.tile([128, 4], f32, tag="bqk")
        bvb_sb = consts.tile([128, CW], bf16, tag="bvb")
        nc.sync.dma_start(out=wq_sb, in_=wq_r)
        nc.scalar.dma_start(out=wk_sb, in_=wk_r)
        nc.scalar.dma_start(out=bqk_sb, in_=aps["bqk"])
        nc.sync.dma_start(out=st_sb, in_=aps["st"])
        nc.scalar.dma_start(out=cos_sb, in_=aps["cosT"])
        nc.sync.dma_start(out=sin_sb, in_=aps["sinT"])
        nc.scalar.dma_start(out=wv_sb, in_=wv_r)
        nc.sync.dma_start(out=bvb_sb, in_=aps["bvb"])
        nc.sync.dma_start(out=wo_sb, in_=wo_r)
        nc.scalar.dma_start(out=ones_sb, in_=aps["ones"])

        def body(_=None):
            qTs, kTs, vts, oTs = {}, {}, {}, {}
            lp = nc.allow_low_precision(
                reason="softmax denominator in bf16; rel tolerance is 2e-2"
            )
            lp.__enter__()

            def emit_A_tb(b, tbl):
                if tbl == 0:
                    qTs[b] = big.tile([128, HPC, S], bf16, tag="qT", name=f"qT{b}")
                    kTs[b] = big.tile([128, HPC, S], bf16, tag="kT", name=f"kT{b}")
                    vts[b] = big.tile([128, NKT, CW], bf16, tag="vtok", name=f"vt{b}")
                qT, kT, vtok = qTs[b], kTs[b], vts[b]
                g0 = b * S + tbl * TB
                s0 = tbl * TB
                xt_t = xtp.tile([128, NCH, TB], f32r, tag="xt")
                xeng = nc.sync if tbl % 2 == 0 else nc.scalar
                xeng.dma_start(out=xt_t, in_=xt_r[:, :, g0 : g0 + TB])

                def emit_rope(strt, dstT, h):
                    tps = psnp.tile([128, TB], f32, tag="psn")
                    nc.tensor.matmul(tps, lhsT=st_sb, rhs=strt,
                                     start=True, stop=True,
                                     skip_group_check=True)
                    t1 = rope.tile([128, TB], bf16, tag="rt")
                    nc.vector.tensor_mul(t1, strt, cos_sb[:, s0 : s0 + TB])
                    t2 = rope.tile([128, TB], bf16, tag="rt")
                    nc.vector.tensor_mul(t2, tps, sin_sb[:, s0 : s0 + TB])
                    nc.vector.tensor_add(dstT[:, h, s0 : s0 + TB], t1, t2)

                prev = None
                for h in range(HPC):
                    for qk, w_sb, bcol, scl, dstT in (
                        (0, wq_sb, h, qscale, qT),
                        (1, wk_sb, 2 + h, 1.0, kT),
                    ):
                        ps = psp.tile([128, TB], f32, tag="ps")
                        for c in range(NCH):
                            nc.tensor.matmul(
                                ps,
                                lhsT=w_sb[:, c, h * HD : (h + 1) * HD],
                                rhs=xt_t[:, c, :],
                                start=(c == 0),
                                stop=(c == NCH - 1),
                                skip_group_check=True,
                            )
                        strt = rope.tile([128, TB], bf16, tag="rt")
                        nc.scalar.activation(
                            strt, ps, Act.Identity,
                            bias=bqk_sb[:, bcol : bcol + 1], scale=scl,
                        )
                        # RoPE for the previous unit while this one's QK
                        # matmuls keep PE busy ahead of it.
                        if prev is not None:
                            emit_rope(*prev)
                        prev = (strt, dstT, h)
                emit_rope(*prev)

                for sub in range(TB // 128):
                    psv = psob.tile([128, CW], f32, tag="pso")
                    for c in range(NCH):
                        nc.tensor.matmul(
                            psv,
                            lhsT=xt_t[:, c, sub * 128 : (sub + 1) * 128],
                            rhs=wv_sb[:, c, :],
                            start=(c == 0),
                            stop=(c == NCH - 1),
                            skip_group_check=True,
                        )
                    nc.vector.tensor_add(
                        vtok[:, tbl * (TB // 128) + sub, :], psv, bvb_sb
                    )

            def emit_B_unit_gen(b, h, qb):
                if h == 0 and qb == 0:
                    oTs[b] = big.tile([128, HPC, S], bf16, tag="outT", name=f"oT{b}")
                qT, kT, vtok, outT = qTs[b], kTs[b], vts[b], oTs[b]
                q0 = qb * QB
                expT = expp.tile([128, 6, QB], bf16, tag="expT")
                pso = psob.tile([128, QB], f32, tag="pso")
                acc = small.tile([128, QB], bf16, tag="acc", bufs=3)

                def consume(kt):
                    nc.tensor.matmul(
                        pso,
                        lhsT=vtok[:, kt, h * HD : (h + 1) * HD],
                        rhs=expT[:, kt % 6, :],
                        start=(kt == 0),
                        stop=(kt == NKT - 1),
                        skip_group_check=True,
                    )
                    if kt == 0:
                        nc.vector.tensor_copy(acc, expT[:, kt % 6, :])
                    else:
                        nc.vector.tensor_add(acc, acc, expT[:, kt % 6, :])

                for pk in range(NKT // 2):
                    ps2 = psp.tile([128, 2, QB], f32, tag="ps")
                    for j in range(2):
                        kt = 2 * pk + j
                        nc.tensor.matmul(
                            ps2[:, j, :],
                            lhsT=kT[:, h, kt * 128 : (kt + 1) * 128],
                            rhs=qT[:, h, q0 : q0 + QB],
                            start=True,
                            stop=True,
                            skip_group_check=True,
                        )
                    sl = (2 * pk) % 6
                    nc.scalar.activation(expT[:, sl : sl + 2, :], ps2, Act.Exp)
                    if pk >= 1:
                        consume(2 * pk - 2)
                        consume(2 * pk - 1)
                    yield
                consume(NKT - 2)
                consume(NKT - 1)
                # partition-sum of acc via ones-column matmul, reciprocal off
                # PSUM, then partition-broadcast via ones-row matmul (PE does
                # the cross-partition moves; no gpsimd involved)
                pss = psnp.tile([1, QB], f32, tag="psn", name="pss")
                nc.tensor.matmul(
                    pss, lhsT=ones_sb[:, 0:1], rhs=acc,
                    start=True, stop=True, skip_group_check=True,
                )
                rec = small.tile([1, QB], bf16, tag="rec")
                nc.vector.reciprocal(rec, pss)
                rbc = psnp.tile([128, QB], f32, tag="psn", name="rbc")
                nc.tensor.matmul(
                    rbc, lhsT=ones_sb[0:1, :], rhs=rec,
                    start=True, stop=True, skip_group_check=True,
                )
                rbs = small.tile([128, QB], bf16, tag="rbs")
                nc.scalar.activation(rbs, rbc, Act.Identity)
                nc.vector.tensor_mul(outT[:, h, q0 : q0 + QB], pso, rbs)

            def emit_C_group_gen(b, qb):
                outT = oTs[b]
                for tt in range(qb * NTT, (qb + 1) * NTT):
                    r0 = b * S + tt * 128
                    stage = stagep.tile([128, NNB, QB], bf16, tag="stage")
                    for nb in range(NNB):
                        psn = psnp.tile([128, QB], f32, tag="psn")
                        for h in range(HPC):
                            nc.tensor.matmul(
                                psn,
                                lhsT=outT[:, h, tt * 128 : (tt + 1) * 128],
                                rhs=wo_sb[:, h, nb * QB : (nb + 1) * QB],
                                start=(h == 0),
                                stop=(h == HPC - 1),
                                skip_group_check=True,
                            )
                        if nb == NNB - 1:
                            nc.scalar.activation(
                                stage[:, nb, :], psn, Act.Identity
                            )
                        else:
                            nc.vector.tensor_copy(stage[:, nb, :], psn)
                        if nb == NNB - 1:
                            eng = nc.sync if tt % 2 == 0 else nc.scalar
                            eng.dma_start(
                                out=out_ap[r0 : r0 + 128, :],
                                in_=stage.rearrange("p n q -> p (n q)"),
                            )
                        yield

            NTBB = S // TB  # A blocks per batch (8)
            for b in range(B):
                for tbl in range(NTBB):
                    emit_A_tb(b, tbl)
            if "B" in phases:
                groups = [(b, qb) for b in range(B) for qb in range(NQB)]
                CLAG = 2
                for gi, (b, qb) in enumerate(groups):
                    g0 = emit_B_unit_gen(b, 0, qb)
                    g1 = emit_B_unit_gen(b, 1, qb)
                    cg = (
                        emit_C_group_gen(*groups[gi - CLAG])
                        if gi >= CLAG and "C" in phases
                        else None
                    )
                    for pk in range(NKT // 2):
                        next(g0, None)
                        next(g1, None)
                        if cg is not None:
                            next(cg, None)
                            next(cg, None)
                    for _ in g0:
                        pass
                    for _ in g1:
                        pass
                    if cg is not None:
                        for _ in cg:
                            pass
                if "C" in phases:
                    for b, qb in groups[-CLAG:]:
                        for _ in emit_C_group_gen(b, qb):
                            pass
            if "C" not in phases:
                # dummy output write so the kernel has an observable effect
                for b in range(B):
                    src = oTs[b] if "B" in phases else qTs[b]
                    st_ = stagep.tile([128, NNB, QB], bf16, tag="stage")
                    nc.vector.tensor_copy(st_[:, 0, :], src[:, 0, :QB])
                    nc.sync.dma_start(
                        out=out_ap[b * S : b * S + 128, :QB], in_=st_[:, 0, :]
                    )
            lp.__exit__(None, None, None)

        if repeat == 1:
            body()
        else:
            eng_hints = (
                mybir.EngineType.PE, mybir.EngineType.Activation,
                mybir.EngineType.DVE, mybir.EngineType.SP,
                mybir.EngineType.Pool,
            )

            def unrollable_body(iv0, unroll):
                for i in range(unroll):
                    body(iv0 + i)

            tc.For_i_unrolled_general(
                0, repeat, 1, unrollable_body, max_unroll=1,
                hint_engines=eng_hints,
            )


def _build(repeat=1, phases="ABC"):
    key = ("nc", repeat, phases)
    if key in _CACHE:
        return _CACHE[key]
    import concourse.bacc as bacc
    import concourse.tile as tile
    from concourse import mybir

    f32 = mybir.dt.float32
    f32r = mybir.dt.float32r
    bf16 = mybir.dt.bfloat16

    nc = bacc.Bacc("TRN2", target_bir_lowering=False, debug=False)
    specs = [
        ("xt", [HIDDEN, T], f32r, "ExternalInput"),
        ("wq", [HIDDEN, CW], f32r, "ExternalInput"),
        ("wk", [HIDDEN, CW], f32r, "ExternalInput"),
        ("wv", [HIDDEN, CW], f32r, "ExternalInput"),
        ("wo", [CW, HIDDEN], bf16, "ExternalInput"),
        ("bqk", [128, 4], f32, "ExternalInput"),
        ("bvb", [128, CW], bf16, "ExternalInput"),
        ("cosT", [128, S], bf16, "ExternalInput"),
        ("sinT", [128, S], bf16, "ExternalInput"),
        ("st", [128, 128], bf16, "ExternalInput"),
        ("ones", [128, 128], bf16, "ExternalInput"),
        ("out", [T, HIDDEN], bf16, "ExternalOutput"),
    ]
    aps = {}
    for name, shape, dt_, kind in specs:
        aps[name] = nc.dram_tensor(name, shape, dt_, kind=kind).ap()
    with tile.TileContext(nc) as tc:
        _kernel_body(tc, aps, repeat=repeat, phases=phases)
    nc.compile()
    _CACHE[key] = nc
    return nc


def _host_inputs(hidden_states, Wq, bq, Wk, bk, Wv, bv, Wo):
    import ml_dtypes

    X = np.ascontiguousarray(
        np.asarray(hidden_states, dtype=np.float32).reshape(T, HIDDEN)
    )
    XT = np.ascontiguousarray(X.T)

    inv = 1.0 / (BASE ** (np.arange(0, HD, 2, dtype=np.float32) / HD))
    t = np.arange(S, dtype=np.float32)
    freqs = np.outer(t, inv)  # [S, 64]
    emb = np.concatenate([freqs, freqs], axis=-1)  # [S, 128]
    cosT = np.ascontiguousarray(np.cos(emb).T.astype(ml_dtypes.bfloat16))  # [128, S]
    sinT = np.ascontiguousarray(np.sin(emb).T.astype(ml_dtypes.bfloat16))

    # S matrix: tmp = S_ @ q gives tmp[p] = -q[p+64] (p<64), q[p-64] (p>=64)
    # matmul computes lhsT.T @ rhs, so pass st = S_^T.
    S_ = np.zeros((128, 128), dtype=np.float32)
    for p in range(64):
        S_[p, p + 64] = -1.0
        S_[p + 64, p] = 1.0
    st = np.ascontiguousarray(S_.T.astype(ml_dtypes.bfloat16))
    ones = np.ones((128, 128), dtype=ml_dtypes.bfloat16)

    in_maps = []
    for c in range(NCORES):
        j0 = c * CW
        bq_c = np.asarray(bq[j0 : j0 + CW], dtype=np.float32)
        bk_c = np.asarray(bk[j0 : j0 + CW], dtype=np.float32)
        bv_c = np.asarray(bv[j0 : j0 + CW], dtype=np.float32)
        # ACT computes in*scale + bias, so pre-scale the q bias columns
        qs = 1.0 / math.sqrt(HD)
        bqk = np.stack(
            [bq_c[:HD] * qs, bq_c[HD:] * qs, bk_c[:HD], bk_c[HD:]], axis=1
        ).astype(np.float32)  # [128, 4]
        in_maps.append(
            {
                "xt": XT,
                "wq": np.ascontiguousarray(Wq[:, j0 : j0 + CW], dtype=np.float32),
                "wk": np.ascontiguousarray(Wk[:, j0 : j0 + CW], dtype=np.float32),
                "wv": np.ascontiguousarray(Wv[:, j0 : j0 + CW], dtype=np.float32),
                "wo": np.ascontiguousarray(np.asarray(Wo[j0 : j0 + CW, :], dtype=np.float32).astype(ml_dtypes.bfloat16)),
                "bqk": np.ascontiguousarray(bqk),
                "bvb": np.ascontiguousarray(
                    np.tile(bv_c[None, :], (128, 1)).astype(ml_dtypes.bfloat16)
                ),
                "cosT": cosT,
                "sinT": sinT,
                "st": st,
                "ones": ones,
            }
        )
    return in_maps


def kernel(hidden_states, Wq, bq, Wk, bk, Wv, bv, Wo):
    from concourse import bass_utils

    nc = _build(repeat=1)
    in_maps = _host_inputs(hidden_states, Wq, bq, Wk, bk, Wv, bv, Wo)
    res = bass_utils.run_bass_kernel_spmd(nc, in_maps, core_ids=list(range(NCORES)))
    acc = res.results[0]["out"].astype(np.float32)
    for c in range(1, NCORES):
        acc = acc + res.results[c]["out"]
    return acc.reshape(B, S, HIDDEN)
